# revision 20
# baseline (speedup 1.0000x reference)
"""GCN (4x GCNConv + global_add_pool + MLP) on 8 Trainium2 NeuronCores.

Sharding: nodes partitioned into 8 contiguous blocks of 12544 (dst
partitioning); each edge is owned by the core that owns its dst.  Per layer
each core computes contrib rows dinv[n]*(h @ W) for its nodes, AllGathers
them into a replicated fp16 table (rows padded to 256B), then pulls per-edge
messages with the gpsimd dma_gather custom instruction (int16 indices =>
the table is addressed in 4 quarters) and scatter-adds them with windowed
one-hot matmuls accumulating in PSUM.

Notes:
- h kept transposed ([96 feat, nodes]): no transposes in the xw matmul or
  the scatter path.
- one-hot is pure 0/1 (single DVE is_equal per tile via a stride-0
  broadcast AP); dinv[dst] is applied to the PSUM result; edges sorted by
  dst => each 128-edge chunk covers a narrow dst window (W columns).
- self-loops are folded into the PSUM init matmul (contribT @ I).
- layer-1 table (x @ W1)*dinv is precomputed on host: no device work and
  no AllGather for layer 1.
- the generic multi-offset indirect_dma_start is avoided: on this stack it
  consumes only the first offset per partition and reads contiguously
  (verified on HW), which is also why the original baseline kernel's
  message passing was silently wrong.
"""

import numpy as np

P = 128
NFEAT = 32
HID = 96
ROW = 128            # table row width (f16) -> 256B, dma_gather granularity
NG = 2048
NCORES = 8
NPC = 12544          # nodes per core (98 * 128)
NT = NPC // P        # 98 node tiles per core
NPAD = NPC * NCORES
NQ = 4               # table quarters (int16 index limit)
QROWS = NPAD // NQ   # 25088 rows per quarter
NG_PAD = NG + 512


def _perm_row(n):
    """Global node id -> row index in the permuted table layout
    (AllGather of per-core [128, NT, ROW] SBUF tensors)."""
    c = n // NPC
    r = n % NPC
    p = r % P
    t = r // P
    return c * NPC + p * NT + t


# ----------------------------------------------------------------------------
# Host-side preprocessing.
# ----------------------------------------------------------------------------

def _prep(x, edge_index, batch, W1, dinv_out):
    N = x.shape[0]
    src = np.asarray(edge_index[0], dtype=np.int64)
    dst = np.asarray(edge_index[1], dtype=np.int64)
    batch = np.asarray(batch, dtype=np.int64)

    deg = np.bincount(dst, minlength=N).astype(np.float64) + 1.0  # + self loop
    dinv = (1.0 / np.sqrt(deg)).astype(np.float32)
    dinv_out[:] = dinv

    owner = dst // NPC
    percore = []
    for c in range(NCORES):
        m = owner == c
        s_c = src[m]
        d_c = dst[m] - c * NPC
        row = _perm_row(s_c)
        q_c = row // QROWS
        rel = row % QROWS
        t_c = d_c // P
        o = np.lexsort((d_c, q_c, t_c))
        d_c, q_c, rel, t_c = d_c[o], q_c[o], rel[o], t_c[o]
        cnt = np.zeros((NT, NQ), np.int64)
        np.add.at(cnt, (t_c, q_c), 1)
        percore.append((d_c, q_c, rel, t_c, cnt))

    # uniform chunk counts per (tile, quarter) across cores
    Mtq = np.ones((NT, NQ), np.int64)
    for c in range(NCORES):
        Mtq = np.maximum(Mtq, -(-percore[c][4] // P))
    ntot = Mtq.sum(axis=1)             # chunks per tile
    cumN = np.zeros(NT + 1, np.int64)
    cumN[1:] = np.cumsum(ntot)
    TOT = int(cumN[-1])                # total chunks
    NTOT_MAX = int(ntot.max())
    MQMX = int(Mtq.max())
    # chunk column index k(t, q, m) = cumN[t] + sum(Mtq[t,:q]) + m
    qoff = np.zeros((NT, NQ), np.int64)
    qoff[:, 1:] = np.cumsum(Mtq, axis=1)[:, :-1]

    # per-core slot assignment + chunk spans
    first = np.full((NCORES, TOT), 1 << 30, np.int64)
    last = np.full((NCORES, TOT), -1, np.int64)
    slotmaps = []
    for c in range(NCORES):
        d_c, q_c, rel, t_c, cnt = percore[c]
        # rank within (t, q) group
        gstart = np.zeros((NT, NQ), np.int64)
        flat = np.cumsum(cnt.reshape(-1))
        gstart.reshape(-1)[1:] = flat[:-1]
        rank = np.arange(len(d_c)) - gstart[t_c, q_c]
        mi = rank // P
        pi = rank % P
        k = cumN[t_c] + qoff[t_c, q_c] + mi
        doff = d_c - t_c * P
        np.minimum.at(first[c], k, doff)
        np.maximum.at(last[c], k, doff)
        slotmaps.append((k, pi, mi, doff, rel, t_c, q_c))

    fmin = first.min(axis=0)
    lmax = last.max(axis=0)
    span = np.where(lmax >= 0, lmax - np.minimum(fmin, lmax) + 1, 1)
    W = max(8, int(-(-int(span.max()) // 8) * 8))
    bases = np.minimum(np.where(fmin > (1 << 29), 0, fmin), P - W)
    bases = np.maximum(bases, 0).astype(np.int64)   # [TOT]
    assert np.all(np.where(lmax >= 0, lmax - bases < W, True))

    COLS = int(Mtq.sum()) * 8          # idx columns (16 rows per 128 idxs)
    colbase = np.zeros((NT, NQ), np.int64)
    colbase.reshape(-1)[1:] = np.cumsum(Mtq.reshape(-1) * 8)[:-1]

    inputs = []
    for c in range(NCORES):
        k, pi, mi, doff, rel, t_c, q_c = slotmaps[c]
        doffp = np.full((P, TOT), -1.0, np.float16)
        doffp[pi, k] = (doff - bases[k]).astype(np.float16)
        idx16 = np.zeros((16, COLS), np.int16)
        col = colbase[t_c, q_c] + (mi * P + pi) // 16
        rowi = (mi * P + pi) % 16
        idx16[rowi, col] = rel.astype(np.int16)
        idx128 = np.tile(idx16, (8, 1))

        n0 = c * NPC
        nreal = min(NPC, max(0, N - n0))
        dloc = np.zeros(NPC, np.float32)
        dloc[:nreal] = dinv[n0 : n0 + nreal]
        gbase = int(batch[n0]) if nreal > 0 else 0
        pg = np.full(NPC, 1.0e9, np.float32)
        pg[:nreal] = (batch[n0 : n0 + nreal] - gbase).astype(np.float32)

        inputs.append(
            dict(
                idx=idx128,
                doffrel=doffp,
                dinv_p=np.ascontiguousarray(dloc.reshape(NT, P).T),
                dinvrow=dloc.reshape(1, NPC).copy(),
                poolg=np.ascontiguousarray(pg.reshape(NT, P).T),
                _gbase=gbase,
                _nreal=nreal,
            )
        )

    # GT uniform
    GT = 1
    for c in range(NCORES):
        n0 = c * NPC
        nreal = inputs[c]["_nreal"]
        if nreal > 0:
            gb = inputs[c]["_gbase"]
            gmax = int(batch[n0 + nreal - 1])
            GT = max(GT, -(-(gmax - gb + 1) // P))
    assert NG + GT * P <= NG_PAD + P
    for c in range(NCORES):
        gbase = inputs[c].pop("_gbase")
        inputs[c].pop("_nreal")
        growidx = (
            gbase
            + np.arange(GT, dtype=np.int32)[None, :] * P
            + np.arange(P, dtype=np.int32)[:, None]
        ).astype(np.int32)
        inputs[c]["growidx"] = growidx

    # host-precomputed layer-1 table in permuted layout: (x @ W1) * dinv
    xw1 = (np.asarray(x, np.float32) @ np.asarray(W1, np.float32)) * dinv[:, None]
    table1 = np.zeros((NPAD, ROW), np.float16)
    rows = _perm_row(np.arange(N, dtype=np.int64))
    table1[rows, :HID] = xw1.astype(np.float16)
    for c in range(NCORES):
        tloc = table1[c * NPC : (c + 1) * NPC]  # rows p*NT + t
        inputs[c]["contrib1"] = np.ascontiguousarray(tloc.reshape(P, NT * ROW))
        inputs[c]["table1"] = table1

    meta = dict(Mtq=Mtq, cumN=cumN, qoff=qoff, bases=bases, W=W, TOT=TOT,
                COLS=COLS, colbase=colbase, MQMX=MQMX, GT=GT,
                ntot=[int(v) for v in ntot])
    return inputs, meta


# ----------------------------------------------------------------------------
# Bass program.
# ----------------------------------------------------------------------------

def _build_program(meta, bf2val):
    from concourse import bacc, bass, mybir, tile

    f32 = mybir.dt.float32
    f16 = mybir.dt.float16
    i32 = mybir.dt.int32
    i16 = mybir.dt.int16
    AF = mybir.ActivationFunctionType
    OP = mybir.AluOpType
    AP = bass.AP

    Mtq, cumN, qoff, bases = meta["Mtq"], meta["cumN"], meta["qoff"], meta["bases"]
    W, TOT, COLS, colbase = meta["W"], meta["TOT"], meta["COLS"], meta["colbase"]
    MQMX, GT, ntot = meta["MQMX"], meta["GT"], meta["ntot"]

    nc = bacc.Bacc("TRN2", target_bir_lowering=False, debug=False)

    tab1_p = nc.declare_dram_parameter("table1", [NPAD, ROW], f16, isOutput=False)
    ctr1_p = nc.declare_dram_parameter("contrib1", [P, NT * ROW], f16, isOutput=False)
    idx_p = nc.declare_dram_parameter("idx", [P, COLS], i16, isOutput=False)
    doff_p = nc.declare_dram_parameter("doffrel", [P, TOT], f16, isOutput=False)
    dinvp_p = nc.declare_dram_parameter("dinv_p", [P, NT], f32, isOutput=False)
    dinvr_p = nc.declare_dram_parameter("dinvrow", [1, NPC], f32, isOutput=False)
    poolg_p = nc.declare_dram_parameter("poolg", [P, NT], f32, isOutput=False)
    grow_p = nc.declare_dram_parameter("growidx", [P, GT], i32, isOutput=False)
    w_ps = [
        nc.declare_dram_parameter(f"w{l}", [HID, HID], f16, isOutput=False)
        for l in (2, 3, 4)
    ]
    b_ps = [
        nc.declare_dram_parameter(f"b{l}", [HID, 1], f32, isOutput=False)
        for l in (1, 2, 3, 4)
    ]
    wf1_p = nc.declare_dram_parameter("wf1", [HID, 32], f32, isOutput=False)
    bf1_p = nc.declare_dram_parameter("bf1", [32, 1], f32, isOutput=False)
    wf2_p = nc.declare_dram_parameter("wf2", [32, 1], f32, isOutput=False)
    out_p = nc.declare_dram_parameter("out", [1, NG], f32, isOutput=True)

    groups = [list(range(NCORES))]

    def expand_w(a, n_inner):
        """AP of `a` ([P, k] slice) with a stride-0 inner dim of n_inner."""
        return AP(a.tensor, a.offset, [list(d) for d in a.ap] + [[0, n_inner]])

    with tile.TileContext(nc) as tc:
        with (
            tc.tile_pool(name="const", bufs=1) as cp,
            tc.tile_pool(name="sb", bufs=1) as sb,
            tc.tile_pool(name="ps", bufs=2, space="PSUM") as ps,
            tc.tile_pool(name="dram", bufs=1, space="DRAM") as dp,
        ):
            # ---- persistent SBUF ---------------------------------------
            hT = cp.tile([HID, NT, P], f16)          # transposed activations
            contrib = cp.tile([P, NT, HID], f16)     # this layer's table rows
            idx_sb = cp.tile([P, COLS], i16)
            doff_sb = cp.tile([P, TOT], f16)
            dinvp_sb = cp.tile([P, NT], f32)
            dinvr_sb = cp.tile([1, NPC], f32)
            dinvB = cp.tile([HID, NT * P], f16)      # dinv broadcast to 96 rows
            poolg_sb = cp.tile([P, NT], f32)
            grow_sb = cp.tile([P, GT], i32)
            w_sb = [cp.tile([HID, HID], f16, name=f"w{l}") for l in (2, 3, 4)]
            b_sb = [cp.tile([HID, 1], f32, name=f"b{l}") for l in (1, 2, 3, 4)]
            wf1_sb = cp.tile([HID, 32], f32)
            bf1_sb = cp.tile([32, 1], f32)
            wf2_sb = cp.tile([32, 1], f32)
            iwk_i = cp.tile([P, max(ntot) * W], i32)
            iwk = cp.tile([P, max(ntot) * W], f16)   # repeating 0..W-1
            iog_i = cp.tile([P, GT * P], i32)
            iog = cp.tile([P, GT * P], f32)
            iop_i = cp.tile([P, P], i32)
            iop = cp.tile([P, P], f32)
            pid_i = cp.tile([P, 1], i32)
            pid = cp.tile([P, 1], f32)
            id16 = cp.tile([P, P], f16)
            id32 = cp.tile([P, P], f32)
            ones1 = cp.tile([1, HID], f32)
            zero_sb = cp.tile([P, HID], f32)
            gsumT = cp.tile([HID, NG], f32)
            zT = cp.tile([32, NG], f32)
            osb = cp.tile([1, NG], f32)

            # ---- DRAM scratch ------------------------------------------
            contribL = dp.tile([P, NT * ROW], f16)
            tables = {
                1: dp.tile([NPAD, ROW], f16, name="table1i", addr_space="Shared"),
                2: dp.tile([NPAD, ROW], f16, name="table2", addr_space="Shared"),
                3: dp.tile([NPAD, ROW], f16, name="table3", addr_space="Shared"),
                4: dp.tile([NPAD, ROW], f16, name="table4", addr_space="Shared"),
            }
            gin = dp.tile([NG_PAD, HID], f32)
            gout = dp.tile([NG_PAD, HID], f32, addr_space="Shared")

            # ---- load constants ----------------------------------------
            nc.sync.dma_start(out=tables[1][:], in_=tab1_p[:])
            nc.sync.dma_start(out=contrib[:], in_=ctr1_p[:].rearrange(
                "p (t f) -> p t f", t=NT)[:, :, 0:HID])
            nc.sync.dma_start(out=idx_sb[:], in_=idx_p[:])
            nc.sync.dma_start(out=doff_sb[:], in_=doff_p[:])
            nc.sync.dma_start(out=dinvp_sb[:], in_=dinvp_p[:])
            nc.sync.dma_start(out=dinvr_sb[:], in_=dinvr_p[:])
            nc.sync.dma_start(out=poolg_sb[:], in_=poolg_p[:])
            nc.sync.dma_start(out=grow_sb[:], in_=grow_p[:])
            for i in range(3):
                nc.sync.dma_start(out=w_sb[i][:], in_=w_ps[i][:])
            for i in range(4):
                nc.sync.dma_start(out=b_sb[i][:], in_=b_ps[i][:])
            nc.sync.dma_start(out=wf1_sb[:], in_=wf1_p[:])
            nc.sync.dma_start(out=bf1_sb[:], in_=bf1_p[:])
            nc.sync.dma_start(out=wf2_sb[:], in_=wf2_p[:])

            nc.gpsimd.iota(iwk[:], pattern=[[0, max(ntot)], [1, W]], base=0,
                           channel_multiplier=0,
                           allow_small_or_imprecise_dtypes=True)
            nc.gpsimd.iota(iog[:], pattern=[[1, GT * P]], base=0,
                           channel_multiplier=0,
                           allow_small_or_imprecise_dtypes=True)
            nc.gpsimd.iota(iop[:], pattern=[[1, P]], base=0,
                           channel_multiplier=0,
                           allow_small_or_imprecise_dtypes=True)
            nc.gpsimd.iota(pid[:], pattern=[[0, 1]], base=0,
                           channel_multiplier=1,
                           allow_small_or_imprecise_dtypes=True)
            nc.vector.tensor_scalar(out=id16[:], in0=iop[:], scalar1=pid[:, 0:1],
                                    scalar2=None, op0=OP.is_equal)
            nc.vector.tensor_scalar(out=id32[:], in0=iop[:], scalar1=pid[:, 0:1],
                                    scalar2=None, op0=OP.is_equal)
            nc.vector.memset(ones1[:], 1.0)
            nc.vector.memset(zero_sb[:], 0.0)
            nc.vector.memset(osb[:], 0.0)

            # dinvB[f, n] = dinv[n] via ones ⊗ dinv chunks
            for j0 in range(0, NPC, 512):
                w_ = min(512, NPC - j0)
                pdv = ps.tile([HID, 512], f32, tag="dv")
                nc.tensor.matmul(out=pdv[:, :w_], lhsT=ones1[:],
                                 rhs=dinvr_sb[:, j0 : j0 + w_],
                                 start=True, stop=True)
                nc.any.tensor_copy(out=dinvB[:, j0 : j0 + w_], in_=pdv[:, :w_])

            # zero the pool scatter buffer
            for r in range(NG_PAD // P):
                nc.sync.dma_start(out=gin[r * P : (r + 1) * P, :], in_=zero_sb[:])

            # ---- 4 GCN layers ------------------------------------------
            for l in (1, 2, 3, 4):
                table = tables[l]
                last = l == 4

                def gather(t):
                    msgs = []
                    for q in range(NQ):
                        n = int(Mtq[t][q]) * P
                        msg = sb.tile([P, MQMX, ROW], f16, tag="msg", bufs=8)
                        cb = int(colbase[t][q])
                        nc.gpsimd.dma_gather(
                            out_ap=msg[:, : Mtq[t][q], :],
                            in_ap=table[q * QROWS : (q + 1) * QROWS, :],
                            idxs_ap=idx_sb[:, cb : cb + n // 16],
                            num_idxs=n, num_idxs_reg=n, elem_size=ROW,
                            single_packet=False)
                        msgs.append(msg)
                    return msgs

                def build_oh(t):
                    n = ntot[t]
                    oh = sb.tile([P, max(ntot) * W], f16, tag="oh", bufs=2)
                    out_ap = AP(oh[:].tensor, oh[:].offset,
                                [list(oh[:].ap[0]), [W, n], [1, W]])
                    in0_ap = AP(iwk[:].tensor, iwk[:].offset,
                                [list(iwk[:].ap[0]), [W, n], [1, W]])
                    nc.vector.tensor_tensor(
                        out=out_ap, in0=in0_ap,
                        in1=expand_w(doff_sb[:, cumN[t] : cumN[t] + n], W),
                        op=OP.is_equal,
                    )
                    return oh

                msgs = {0: gather(0)}
                ohs = {0: build_oh(0)}
                for t in range(NT):
                    if t + 1 < NT:
                        msgs[t + 1] = gather(t + 1)
                        ohs[t + 1] = build_oh(t + 1)
                    msg4, oh = msgs.pop(t), ohs.pop(t)
                    pacc = ps.tile([HID, P], f32, tag="acc", bufs=2)
                    # self-loop init: pacc = contribT (carries dinv^1; the
                    # post dinv multiply makes it dinv^2 * xw)
                    nc.tensor.matmul(out=pacc[:], lhsT=contrib[:, t, :],
                                     rhs=id16[:], start=True, stop=False)
                    nk = ntot[t]
                    ki = 0
                    for q in range(NQ):
                        for m in range(int(Mtq[t][q])):
                            b = int(bases[cumN[t] + ki])
                            nc.tensor.matmul(
                                out=pacc[:, b : b + W],
                                lhsT=msg4[q][:, m, :HID],
                                rhs=oh[:, ki * W : (ki + 1) * W],
                                start=False, stop=(ki == nk - 1),
                                skip_group_check=True,
                            )
                            ki += 1
                    # hT = relu(dinv_dst * pacc + bias)
                    nc.vector.tensor_tensor(
                        out=hT[:, t, :], in0=pacc[:],
                        in1=dinvB[:, t * P : (t + 1) * P], op=OP.mult)
                    nc.scalar.activation(
                        out=hT[:, t, :], in_=hT[:, t, :], func=AF.Relu,
                        bias=b_sb[l - 1][:, 0:1], scale=1.0)
                    # lagged by one tile: next layer's contrib / pool copy
                    if t >= 1:
                        tt = t - 1
                        if not last:
                            pxw = ps.tile([P, HID], f32, tag="xw", bufs=2)
                            nc.tensor.matmul(out=pxw[:], lhsT=hT[:, tt, :],
                                             rhs=w_sb[l - 1][:],
                                             start=True, stop=True)
                            nc.vector.tensor_scalar(
                                out=contrib[:, tt, :], in0=pxw[:],
                                scalar1=dinvp_sb[:, tt : tt + 1],
                                scalar2=None, op0=OP.mult)
                        else:
                            ptr = ps.tile([P, HID], f16, tag="tr", bufs=2)
                            nc.tensor.transpose(out=ptr[:], in_=hT[:, tt, :],
                                                identity=id16[0:96, 0:96])
                            nc.any.tensor_copy(out=contrib[:, tt, :],
                                               in_=ptr[:])
                tt = NT - 1
                if not last:
                    pxw = ps.tile([P, HID], f32, tag="xw", bufs=2)
                    nc.tensor.matmul(out=pxw[:], lhsT=hT[:, tt, :],
                                     rhs=w_sb[l - 1][:], start=True, stop=True)
                    nc.vector.tensor_scalar(
                        out=contrib[:, tt, :], in0=pxw[:],
                        scalar1=dinvp_sb[:, tt : tt + 1],
                        scalar2=None, op0=OP.mult)
                    nc.sync.dma_start(
                        out=contribL[:].rearrange(
                            "p (t r) -> p t r", r=ROW)[:, :, 0:HID],
                        in_=contrib[:])
                    nc.gpsimd.collective_compute(
                        "AllGather", OP.bypass, replica_groups=groups,
                        ins=[contribL[:]], outs=[tables[l + 1][:]])
                else:
                    ptr = ps.tile([P, HID], f16, tag="tr", bufs=2)
                    nc.tensor.transpose(out=ptr[:], in_=hT[:, tt, :],
                                        identity=id16[0:96, 0:96])
                    nc.any.tensor_copy(out=contrib[:, tt, :], in_=ptr[:])

            # ---- global_add_pool (contrib now holds h in [node, feat]) --
            for g in range(GT):
                pg_ps = ps.tile([P, HID], f32, tag="xw", bufs=2)
                for t in range(NT):
                    ohp = sb.tile([P, P], f16, tag="ohp", bufs=4)
                    nc.vector.tensor_scalar(
                        out=ohp[:], in0=iog[:, g * P : (g + 1) * P],
                        scalar1=poolg_sb[:, t : t + 1], scalar2=None,
                        op0=OP.is_equal)
                    nc.tensor.matmul(out=pg_ps[:], lhsT=ohp[:],
                                     rhs=contrib[:, t, :],
                                     start=(t == 0), stop=(t == NT - 1))
                gsb = sb.tile([P, HID], f32, tag="gsb", bufs=2)
                nc.any.tensor_copy(out=gsb[:], in_=pg_ps[:])
                nc.gpsimd.indirect_dma_start(
                    out=gin[:],
                    out_offset=bass.IndirectOffsetOnAxis(
                        ap=grow_sb[:, g : g + 1], axis=0),
                    in_=gsb[:], in_offset=None)

            nc.gpsimd.collective_compute(
                "AllReduce", OP.add, replica_groups=groups,
                ins=[gin[:]], outs=[gout[:]])

            # transpose g back: [NG, HID] -> [HID, NG]
            for j in range(NG // P):
                grow_t = sb.tile([P, HID], f32, tag="gsb", bufs=2)
                nc.sync.dma_start(out=grow_t[:],
                                  in_=gout[j * P : (j + 1) * P, :])
                pT = ps.tile([HID, P], f32, tag="dv")
                nc.tensor.transpose(out=pT[:], in_=grow_t[:], identity=id32[:])
                nc.any.tensor_copy(out=gsumT[:, j * P : (j + 1) * P], in_=pT[:])

            # ---- MLP head ----------------------------------------------
            for j in range(NG // 512):
                pz = ps.tile([32, 512], f32, tag="dv")
                nc.tensor.matmul(out=pz[:], lhsT=wf1_sb[:],
                                 rhs=gsumT[:, j * 512 : (j + 1) * 512],
                                 start=True, stop=True)
                nc.scalar.activation(out=zT[:, j * 512 : (j + 1) * 512],
                                     in_=pz[:], func=AF.Relu,
                                     bias=bf1_sb[:, 0:1], scale=1.0)
                po = ps.tile([1, 512], f32, tag="dv")
                nc.tensor.matmul(out=po[:], lhsT=wf2_sb[:],
                                 rhs=zT[:, j * 512 : (j + 1) * 512],
                                 start=True, stop=True)
                nc.vector.tensor_scalar(
                    out=osb[:, j * 512 : (j + 1) * 512], in0=po[:],
                    scalar1=float(bf2val), scalar2=None, op0=OP.add)
            nc.sync.dma_start(out=out_p[:], in_=osb[:])

    nc.finalize()
    return nc


# ----------------------------------------------------------------------------
# Entry point.
# ----------------------------------------------------------------------------

_RUN_KWARGS = {}
_LAST_RES = None


def kernel(
    x, edge_index, batch,
    W1, b1, W2, b2, W3, b3, W4, b4, Wf1, bf1, Wf2, bf2,
):
    from concourse.bass_utils import run_bass_kernel_spmd

    dinv = np.zeros(np.asarray(x).shape[0], np.float32)
    inputs, meta = _prep(np.asarray(x), np.asarray(edge_index),
                         np.asarray(batch), np.asarray(W1), dinv)
    bf2val = float(np.asarray(bf2).reshape(-1)[0])
    nc = _build_program(meta, bf2val)

    shared = dict(
        w2=np.asarray(W2, np.float16),
        w3=np.asarray(W3, np.float16),
        w4=np.asarray(W4, np.float16),
        b1=np.asarray(b1, np.float32).reshape(HID, 1),
        b2=np.asarray(b2, np.float32).reshape(HID, 1),
        b3=np.asarray(b3, np.float32).reshape(HID, 1),
        b4=np.asarray(b4, np.float32).reshape(HID, 1),
        wf1=np.asarray(Wf1, np.float32),
        bf1=np.asarray(bf1, np.float32).reshape(32, 1),
        wf2=np.asarray(Wf2, np.float32).reshape(32, 1),
    )
    in_maps = [{**inputs[c], **shared} for c in range(NCORES)]
    res = run_bass_kernel_spmd(
        nc, in_maps, core_ids=list(range(NCORES)), **_RUN_KWARGS
    )
    global _LAST_RES
    _LAST_RES = res
    out = np.asarray(res.results[0]["out"]).reshape(NG, 1).astype(np.float32)
    return out
